# revision 22
# baseline (speedup 1.0000x reference)
"""Distributed Trainium2 kernel for 8-head MHA with axial (2D) RoPE.

Problem: x:(2,4096,512) f32, Wq/Wk/Wv/Wo:(512,512), T=128, V=32.
  q/k/v = x @ W.T split into 8 heads of 64; q,k get axial rope
  (first 32 chans rotated by angle t_idx=s//V, next 32 by v_idx=s%V,
  interleaved-pair convention); dense softmax attention; out proj.

Sharding (8 cores): core c -> batch b=c//4, head pair (2*(c%4), 2*(c%4)+1).
Each core computes the full attention for its two heads and a partial
output projection over its 128 channels; the host sums the 4 partials
per batch.

Per-core schedule (matmuls bf16, accumulation f32): the ScalarE exp
stream (256 instructions x ~1004 ns, one per 128-key x 512-query tile
covering both heads) is the bottleneck resource, so everything else is
arranged around keeping it saturated from ~15 us onward:
  - projections+rope are software-pipelined INTO the attention tile
    loop (chunk-granular work items paced against their QK/PV/chain
    deadlines) instead of running as a 45 us prologue.
  - rope: roped = q*cos + swap(q)*sin_signed where swap(q) is computed
    as a SECOND projection with pair-permuted weight rows (host-baked),
    so the whole rope is 3 DVE tensor-tensor ops reading both psum
    tiles directly -- no cast, no strided swap DMA on the chain.
    All element-wise work stays on DVE: Pool-engine tensor ops were
    measured to slow every cross-engine wait by ~200ns (event-accel
    degradation), so gpsimd only drives bulk SWDGE input DMAs.
  - attention: scores^T = k^T.T @ q^T per 128-key tile (both heads in
    one PE pass via row tile_position), exp on ScalarE straight from
    PSUM (scale=1/8 fused), PV matmul with a ones column appended to V
    so PSUM row 64 accumulates the softmax denominator. QK for tile
    t+2 is emitted before PV of tile t. ep tiles are 5-deep: Tile's
    per-engine counting sems make EXP(g+k) wait on PV(g) via the ep
    ring WAR, so a 3-deep ring let the PV tail pace the exp stream.
  - denominators transpose via a DRAM bounce whose latency hides under
    the next chunk (final chunk: PE-mode transposes); reciprocal;
    applied per-partition after the out projection (the last chunk's
    scale runs on the then-idle ScalarE).
  - input DMA routing avoids queue convoys: sync carries the small
    startup-critical loads then the steady-state small DMAs; scalar
    carries three mid-priority loads (pre-exp only); gpsimd (SWDGE)
    streams the bulk late-chunk inputs in deadline order.
"""

import numpy as np
import ml_dtypes

B, S, D, H, HD = 2, 4096, 512, 8, 64
ROT_T = ROT_V = 32
ROPE_BASE = 10000.0
NCORES = 8
P = 128
CHUNK = 512  # sq chunk (one psum bank wide)

_cache = {}


def _install_drain_patch():
    """This walrus build allows only one sync-wait on a CTRL instruction;
    Tile's tail drain carries one wait per live semaphore. Move the extra
    waits onto dedicated SP nops."""
    import concourse.tile as tile
    import concourse.mybir as mybir
    from concourse.tile import ScopedClock

    if getattr(tile.TileContext, "_drain_patch_installed", False):
        return

    def _drain_and_barrier(self, tick_clock, wait_clock):
        nc = self.nc
        drain_inst = nc.sync.drain()
        wait_clock.add_sem_waits(
            drain_inst.ins, ScopedClock({None: tick_clock.global_clock})
        )
        si = drain_inst.ins.sync_info
        ow = list(si.on_wait or [])
        if len(ow) > 1:
            si.on_wait = [ow[0]]
            for w in ow[1:]:
                nop = nc.sync.nop(nofuse=True)
                nop.ins.sync_info = mybir.SyncInfo(on_wait=[w], on_update=[])
        nc.all_engine_barrier()
        popped = nc._tile_sem_poison_stack.pop()
        assert popped is self._sem_poison
        nc.clear_and_free_semaphores(list(self.sems.allocated().values()))
        nc.all_engine_barrier()

    tile.TileContext._drain_and_barrier = _drain_and_barrier
    tile.TileContext._drain_patch_installed = True


def _split_multiwaits(nc):
    """core_v3 walrus allows a single sync-wait command per instruction.
    Hoist extra waits onto same-engine NOPs inserted just before."""
    import concourse.mybir as mybir

    for f in nc.m.functions:
        for blk in f.blocks:
            new = []
            changed = False
            for ins in blk.instructions:
                si = getattr(ins, "sync_info", None)
                ow = list(si.on_wait) if (si is not None and si.on_wait) else []
                eng = getattr(ins, "engine", None)
                if len(ow) > 1 and eng is not None:
                    for i, w in enumerate(ow[:-1]):
                        new.append(
                            mybir.InstNoOp(
                                name=f"{ins.name}-sw{i}",
                                engine=eng,
                                sync_info=mybir.SyncInfo(
                                    on_wait=[w], on_update=[]
                                ),
                                bass_nofuse=True,
                            )
                        )
                    si.on_wait = [ow[-1]]
                    changed = True
                new.append(ins)
            if changed:
                blk.instructions = new


def _build(s_len):
    import concourse.bass as bass
    import concourse.tile as tile
    import concourse.mybir as mybir
    from concourse.bass import ds, ts

    _install_drain_patch()

    f32 = mybir.dt.float32
    bf16 = mybir.dt.bfloat16
    i32 = mybir.dt.int32
    NT = s_len // P        # 128-row tiles (also key tiles)
    NCH = s_len // CHUNK   # 512-wide query chunks
    TPC = CHUNK // P       # s-tiles per chunk
    NG = NCH * NT          # total (chunk, key-tile) iterations

    nc = bass.Bass()
    xT = nc.dram_tensor("xT", [P, 4, s_len], bf16, kind="ExternalInput")
    wqT = nc.dram_tensor("wqT", [P, 4, P], bf16, kind="ExternalInput")
    wqPT = nc.dram_tensor("wqPT", [P, 4, P], bf16, kind="ExternalInput")
    wkT = nc.dram_tensor("wkT", [P, 4, P], bf16, kind="ExternalInput")
    wvT = nc.dram_tensor("wvT", [P, 4, P], bf16, kind="ExternalInput")
    woT = nc.dram_tensor("woT", [HD, 2, D], bf16, kind="ExternalInput")
    ctab = nc.dram_tensor("ctab", [P, s_len], bf16, kind="ExternalInput")
    stab = nc.dram_tensor("stab", [P, s_len], bf16, kind="ExternalInput")
    yp = nc.dram_tensor("yp", [s_len, D], f32, kind="ExternalOutput")

    with tile.TileContext(nc) as tc:
        with (
            tc.tile_pool(name="const", bufs=1) as cpool,
            tc.tile_pool(name="rope", bufs=6) as rope,
            tc.tile_pool(name="vstg", bufs=3) as vstg,
            tc.tile_pool(name="expp", bufs=8) as expp,
            tc.tile_pool(name="schp", bufs=2) as schp,
            tc.tile_pool(name="dch", bufs=2) as dch,
            tc.tile_pool(name="outs", bufs=3) as outs,
            tc.tile_pool(name="qkps", bufs=2, space="PSUM") as qkps,
            tc.tile_pool(name="pvps", bufs=2, space="PSUM") as pvps,
            tc.tile_pool(name="prps", bufs=2, space="PSUM") as prps,
            tc.tile_pool(name="dram", bufs=1, space="DRAM") as dram,
        ):
            mul = mybir.AluOpType.mult
            add = mybir.AluOpType.add
            exp_f = mybir.ActivationFunctionType.Exp
            copy_f = mybir.ActivationFunctionType.Copy
            scale = HD ** -0.5

            # ---- exp-table preload: a dummy 2-element exp at t=0 pulls
            # the ~1.3us ACT_TABLE_LOAD off the critical path ----
            tiny = cpool.tile([1, 2], f32)
            nc.vector.memset(tiny[:], 0.0)
            tinyo = cpool.tile([1, 2], bf16)
            nc.scalar.activation(tinyo[:], tiny[:], exp_f, scale=1.0)

            # ---- input DMAs (see module docstring for queue policy).
            # xc0/xc1 split per dt-slice so the projection matmuls start
            # as each slice lands (Tile tracks per-slice deps). ----
            wk_sb = cpool.tile([P, 4, P], bf16)
            nc.sync.dma_start(wk_sb[:], wkT[:])
            xT_sb = cpool.tile([P, 4, s_len], bf16)
            for dt in range(4):
                nc.sync.dma_start(
                    xT_sb[:, dt, ts(0, CHUNK)], xT[:, dt, ts(0, CHUNK)]
                )
            ct_sb = cpool.tile([P, s_len], bf16)
            st_sb = cpool.tile([P, s_len], bf16)
            nc.sync.dma_start(ct_sb[:, ts(0, CHUNK)], ctab[:, ts(0, CHUNK)])
            nc.sync.dma_start(st_sb[:, ts(0, CHUNK)], stab[:, ts(0, CHUNK)])
            wv_sb = cpool.tile([P, 4, P], bf16)
            nc.sync.dma_start(wv_sb[:], wvT[:])
            wq_sb = cpool.tile([P, 4, P], bf16)
            nc.sync.dma_start(wq_sb[:], wqT[:])
            wqP_sb = cpool.tile([P, 4, P], bf16)
            nc.sync.dma_start(wqP_sb[:], wqPT[:])
            nc.scalar.dma_start(
                xT_sb[:, :, ts(1, CHUNK)], xT[:, :, ts(1, CHUNK)]
            )
            nc.scalar.dma_start(ct_sb[:, ts(1, CHUNK)], ctab[:, ts(1, CHUNK)])
            nc.scalar.dma_start(st_sb[:, ts(1, CHUNK)], stab[:, ts(1, CHUNK)])
            wo_sb = cpool.tile([HD, 2, D], bf16)
            for c in range(2, NCH):
                nc.gpsimd.dma_start(ct_sb[:, ts(c, CHUNK)], ctab[:, ts(c, CHUNK)])
                nc.gpsimd.dma_start(st_sb[:, ts(c, CHUNK)], stab[:, ts(c, CHUNK)])
                nc.gpsimd.dma_start(
                    xT_sb[:, :, ts(c, CHUNK)], xT[:, :, ts(c, CHUNK)]
                )
            nc.gpsimd.dma_start(wo_sb[:], woT[:])

            # ---- persistent tiles ----
            qT = cpool.tile([P, s_len], bf16)   # roped q^T
            kT = cpool.tile([P, s_len], bf16)
            v_sb = cpool.tile([P, NT, 2, HD + 1], bf16)  # v natural + ones
            yT0 = cpool.tile([HD, s_len], bf16)
            yT1 = cpool.tile([HD, s_len], bf16)
            den_dram = dram.tile([2, s_len], f32)

            nc.vector.memset(v_sb[:, :, :, HD : HD + 1], 1.0)
            one64 = cpool.tile([HD + 1, 1], f32)
            nc.vector.memset(one64[:], 1.0)

            # ---- chunk-granular projection + rope units ----
            def proj_mms(w_sb, c, name):
                ps = prps.tile([P, CHUNK], f32, tag="pr", name=name)
                for dt in range(4):
                    nc.tensor.matmul(
                        ps[:],
                        lhsT=w_sb[:, dt, :],
                        rhs=xT_sb[:, dt, ts(c, CHUNK)],
                        start=(dt == 0),
                        stop=(dt == 3),
                    )
                return ps

            def kq_perm(w_sb, wP_sb, dst, c):
                """One chunk of projection + axial rope via two projections
                (normal + pair-permuted weights), then dst = ps*cos +
                psP*sin_signed as 3 DVE ops from PSUM. PE-heavy (8 MMs)
                but short-chain -- used for q whose deadlines are spread."""
                sl = ts(c, CHUNK)
                ps = proj_mms(w_sb, c, "pj")
                m1 = rope.tile([P, CHUNK], f32, tag="rope")
                psP = proj_mms(wP_sb, c, "pjP")
                nc.vector.tensor_tensor(m1[:], ps[:], ct_sb[:, sl], mul)
                m2 = rope.tile([P, CHUNK], f32, tag="rope")
                nc.vector.tensor_tensor(m2[:], psP[:], st_sb[:, sl], mul)
                nc.vector.tensor_tensor(dst[:, sl], m1[:], m2[:], add)

            def kq_swap(w_sb, dst, c):
                """One chunk of projection + axial rope via the strided
                pair-swap DMA. PE-light (4 MMs) -- used for k, whose
                chunks are all due within the first attention chunk."""
                sl = ts(c, CHUNK)
                ps = proj_mms(w_sb, c, "pj")
                p0 = rope.tile([P, CHUNK], bf16, tag="p0")
                ps0 = rope.tile([P, CHUNK], bf16, tag="p0")
                nc.vector.tensor_copy(p0[:], ps[:])
                nc.sync.dma_start(ps0[0:P:2, :], p0[1:P:2, :])
                nc.sync.dma_start(ps0[1:P:2, :], p0[0:P:2, :])
                nc.vector.tensor_tensor(dst[:, sl], p0[:], ct_sb[:, sl], mul)
                nc.vector.tensor_tensor(ps0[:], ps0[:], st_sb[:, sl], mul)
                nc.vector.tensor_tensor(dst[:, sl], dst[:, sl], ps0[:], add)

            def v_chunk(c):
                ps = proj_mms(wv_sb, c, "pjv")
                vs = vstg.tile([P, CHUNK], bf16, tag="vs")
                nc.vector.tensor_copy(vs[:], ps[:])
                vn = vstg.tile([P, TPC, P], bf16, tag="vn")
                nc.sync.dma_start(vn[:], vs[:], transpose=True)
                nc.vector.tensor_copy(
                    v_sb[:, c * TPC : (c + 1) * TPC, :, 0:HD],
                    vn.rearrange("p j (h c) -> p j h c", h=2),
                )

            # ---- attention helpers ----
            qks = {}
            eps = {}

            def offloaded(g):
                c, t = divmod(g, NT)
                return c >= 1 and t >= 6 and t % 5 == 2

            def emit_qk(g):
                if g >= NG:
                    return
                c, t = divmod(g, NT)
                qk = qkps.tile([P, 2, CHUNK], f32, tag="qk")
                nc.tensor.matmul(
                    qk[:, 0, :],
                    lhsT=kT[0:HD, ts(t, P)],
                    rhs=qT[0:HD, ts(c, CHUNK)],
                    start=True,
                    stop=True,
                    tile_position=(0, 0),
                )
                nc.tensor.matmul(
                    qk[:, 1, :],
                    lhsT=kT[HD:P, ts(t, P)],
                    rhs=qT[HD:P, ts(c, CHUNK)],
                    start=True,
                    stop=True,
                    tile_position=(HD, 0),
                )
                qks[g] = qk
                if offloaded(g):
                    # Schraudolph exp on DVE, computed in the 2-pair
                    # prefetch shadow so PV(g) never waits on it:
                    # exp(s/8) ~= bitcast of int32(A*s + B). A folds the
                    # 1/8 score scale and log2(e); B carries the 127
                    # exponent bias minus the mantissa-linear correction
                    # (C=486411, rel err ~2%, mostly cancelling in the
                    # softmax ratio). Offloads ~14% of the exp stream
                    # off the bottleneck ScalarE.
                    ti = schp.tile([P, 2, CHUNK], i32, tag="ti")
                    nc.vector.tensor_scalar(
                        ti[:], qk[:], 1512775.40, 1064866805.0, mul, add
                    )
                    ep = expp.tile([P, 2, CHUNK], bf16, tag="ep")
                    nc.vector.tensor_copy(ep[:], ti[:].bitcast(f32))
                    eps[g] = ep

            def recip_of(rrs):
                rr = dch.tile([P, 2, TPC], f32, tag="rr")
                nc.vector.reciprocal(rr[:], rrs)
                return rr

            def outproj_st(c, rr, st, tail=False):
                """One s-tile of the deferred output projection of chunk c.
                In the tail (after the last exp) ScalarE is free and takes
                one of the two per-partition scales."""
                sg = c * TPC + st
                op0 = prps.tile([P, CHUNK], f32, tag="pr")
                nc.tensor.matmul(
                    op0[:],
                    lhsT=yT0[:, ts(sg, P)],
                    rhs=wo_sb[:, 0, :],
                    start=True,
                    stop=True,
                )
                acc = outs.tile([P, D], f32, tag="acc")
                if tail:
                    nc.scalar.activation(
                        acc[:], op0[:], copy_f, scale=rr[:, 0, st : st + 1]
                    )
                else:
                    nc.vector.tensor_scalar_mul(
                        acc[:], op0[:], rr[:, 0, st : st + 1]
                    )
                op1 = prps.tile([P, CHUNK], f32, tag="pr")
                nc.tensor.matmul(
                    op1[:],
                    lhsT=yT1[:, ts(sg, P)],
                    rhs=wo_sb[:, 1, :],
                    start=True,
                    stop=True,
                )
                tmp = outs.tile([P, D], f32, tag="tmp")
                nc.vector.tensor_scalar_mul(tmp[:], op1[:], rr[:, 1, st : st + 1])
                nc.vector.tensor_tensor(acc[:], acc[:], tmp[:], add)
                nc.sync.dma_start(yp[ts(sg, P), :], acc[:])

            # ---- pre-loop: HAM warmup on the first-landed weights,
            # then chunk-0 chains + chunk 1 k/v ----
            wps = prps.tile([P, CHUNK], f32, tag="pr", name="warm")
            for i in range(6):
                nc.tensor.matmul(
                    wps[:],
                    lhsT=wk_sb[:, 0, :],
                    rhs=wk_sb[:, :, :],
                    start=(i == 0),
                    stop=(i == 5),
                )
            kq_swap(wk_sb, kT, 0)
            v_chunk(0)
            kq_perm(wq_sb, wqP_sb, qT, 0)
            emit_qk(0)
            emit_qk(1)
            kq_swap(wk_sb, kT, 1)
            v_chunk(1)

            # ---- paced work items: (chunk, tile) -> emissions.
            # Deadlines: kT c_j by pair 4j-2 (QK prefetch), v c_j by pair
            # 4j (PV), qT c_j by pair 32(j-1)+30; each chain needs ~3-4
            # pairs from emission. ----
            work = {}
            for j in range(2, NCH):
                work.setdefault((0, 4 * (j - 2)), []).append(
                    lambda j=j: kq_swap(wk_sb, kT, j)
                )
                work.setdefault((0, 4 * (j - 2) + 2), []).append(
                    lambda j=j: v_chunk(j)
                )
            for j in range(1, NCH):
                slot = (0, 13) if j == 1 else (j - 1, 16)
                work.setdefault(slot, []).append(
                    lambda j=j: kq_perm(wq_sb, wqP_sb, qT, j)
                )

            # ---- attention main loop. QK prefetched 2 tiles ahead;
            # chunk c-1's output projection interleaved into chunk c's
            # tile loop (reciprocal at tile 6, one s-tile every 2 tiles
            # from tile 8). ----
            spread = NT >= 8 + 2 * TPC
            pend = None
            for c in range(NCH):
                if pend is not None and not spread:
                    rr = recip_of(pend[1])
                    for st in range(TPC):
                        outproj_st(pend[0], rr, st)
                    pend = None
                pv0 = pvps.tile([P, CHUNK], f32, tag="pv")
                pv1 = pvps.tile([P, CHUNK], f32, tag="pv")
                rr = None
                for t in range(NT):
                    g = c * NT + t
                    for thunk in work.pop((c, t), ()):
                        thunk()
                    qk = qks.pop(g)
                    if g in eps:
                        ep = eps.pop(g)
                    else:
                        ep = expp.tile([P, 2, CHUNK], bf16, tag="ep")
                        nc.scalar.activation(ep[:], qk[:], exp_f, scale=scale)
                    emit_qk(g + 2)
                    nc.tensor.matmul(
                        pv0[0 : HD + 1, :],
                        lhsT=v_sb[:, t, 0, :],
                        rhs=ep[:, 0, :],
                        start=(t == 0),
                        stop=(t == NT - 1),
                    )
                    nc.tensor.matmul(
                        pv1[0 : HD + 1, :],
                        lhsT=v_sb[:, t, 1, :],
                        rhs=ep[:, 1, :],
                        start=(t == 0),
                        stop=(t == NT - 1),
                    )
                    if pend is not None and spread:
                        if t == 6:
                            rr = recip_of(pend[1])
                        elif t >= 8 and t % 2 == 0 and (t - 8) // 2 < TPC:
                            outproj_st(pend[0], rr, (t - 8) // 2)
                            if (t - 8) // 2 == TPC - 1:
                                pend = None
                # y^T and denominators out of PSUM; denominator transpose
                # via a DRAM bounce (latency hidden by the deferral).
                # pv0's readers first so its bank frees for chunk c+1.
                dt_sb = dch.tile([HD + 1, 2, CHUNK], f32, tag="den")
                nc.vector.tensor_copy(yT0[:, ts(c, CHUNK)], pv0[0:HD, :])
                nc.vector.tensor_copy(dt_sb[HD : HD + 1, 0, :], pv0[HD : HD + 1, :])
                nc.vector.tensor_copy(yT1[:, ts(c, CHUNK)], pv1[0:HD, :])
                nc.vector.tensor_copy(dt_sb[HD : HD + 1, 1, :], pv1[HD : HD + 1, :])
                if spread and c == NCH - 1:
                    last_dt = dt_sb
                    continue
                nc.sync.dma_start(
                    den_dram[0:1, ts(c, CHUNK)], dt_sb[HD : HD + 1, 0, :]
                )
                nc.sync.dma_start(
                    den_dram[1:2, ts(c, CHUNK)], dt_sb[HD : HD + 1, 1, :]
                )
                rt = dch.tile([P, 2, TPC], f32, tag="rt")
                for h in range(2):
                    nc.sync.dma_start(
                        rt[:, h, :],
                        den_dram[h, ts(c, CHUNK)].rearrange("(t p) -> p t", p=P),
                    )
                pend = (c, rt[:])
            if spread:
                # final chunk: transpose the denominator rows on the PE
                # (no DRAM round-trip on the tail critical path)
                dps = prps.tile([P, 2, TPC], f32, tag="pr")
                for h in range(2):
                    for st in range(TPC):
                        nc.tensor.transpose(
                            dps[:, h, st : st + 1],
                            last_dt[HD : HD + 1, h, ts(st, P)],
                            one64[HD : HD + 1, :],
                        )
                rrl = dch.tile([P, 2, TPC], f32, tag="rr")
                nc.vector.reciprocal(rrl[:], dps[:])
                for st in range(TPC):
                    outproj_st(NCH - 1, rrl, st, tail=True)
            else:
                rr = recip_of(pend[1])
                for st in range(TPC):
                    outproj_st(pend[0], rr, st, tail=True)
    _split_multiwaits(nc)
    return nc


def _host_inputs(x, Wq, Wk, Wv, Wo, V, s_len):
    """Build the 8 per-core input dicts."""
    bf = ml_dtypes.bfloat16
    x = np.asarray(x, np.float32)
    Wq = np.asarray(Wq, np.float32)
    Wk = np.asarray(Wk, np.float32)
    Wv = np.asarray(Wv, np.float32)
    Wo = np.asarray(Wo, np.float32)

    # rope tables in channel-on-partition layout [128, s]:
    #   row h*64+c: cos/sin of the angle for pair f=c//2 (t-axis for c<32,
    #   v-axis for c>=32); sin row signed: -sin even c, +sin odd c.
    s = np.arange(s_len)
    half_t, half_v = ROT_T // 2, ROT_V // 2
    inv_t = 1.0 / (ROPE_BASE ** (np.arange(half_t, dtype=np.float64) / half_t))
    inv_v = 1.0 / (ROPE_BASE ** (np.arange(half_v, dtype=np.float64) / half_v))
    ang_t = (s // V)[:, None] * inv_t[None, :]  # (s, 16)
    ang_v = (s % V)[:, None] * inv_v[None, :]
    ang = np.concatenate([ang_t, ang_v], axis=1)  # (s, 32) per-pair angle
    ang_ch = np.repeat(ang, 2, axis=1)  # (s, 64) per-channel
    sign = np.where(np.arange(HD) % 2 == 0, -1.0, 1.0)[None, :]
    cos_ch = np.cos(ang_ch)  # (s, 64)
    sin_ch = np.sin(ang_ch) * sign
    ctab = np.ascontiguousarray(np.tile(cos_ch, (1, 2)).T).astype(bf)  # [128, s]
    stab = np.ascontiguousarray(np.tile(sin_ch, (1, 2)).T).astype(bf)

    # channel pair swap for the rope cross term, baked into permuted
    # weight rows (swap(q) = x @ Wq[perm].T)
    perm = np.arange(D) ^ 1

    xT = {}
    for b in range(B):
        t = x[b, :s_len].T.reshape(4, P, s_len).transpose(1, 0, 2)
        xT[b] = np.ascontiguousarray(t).astype(bf)

    def wslice(W, sl):
        return np.ascontiguousarray(
            W[sl, :].T.reshape(4, P, P).transpose(1, 0, 2)
        ).astype(bf)

    WqP = Wq[perm]

    ins = []
    for core in range(NCORES):
        b = core // 4
        hb = (core % 4) * 2 * HD
        sl = slice(hb, hb + 2 * HD)
        woT = np.ascontiguousarray(
            Wo[:, sl].T.reshape(2, HD, D).transpose(1, 0, 2)
        ).astype(bf)
        ins.append(
            {
                "xT": xT[b],
                "wqT": wslice(Wq, sl),
                "wqPT": wslice(WqP, sl),
                "wkT": wslice(Wk, sl),
                "wvT": wslice(Wv, sl),
                "woT": woT,
                "ctab": ctab,
                "stab": stab,
            }
        )
    return ins


def kernel(x, Wq, Wk, Wv, Wo, T, V, _trace=False):
    from concourse.bass_utils import run_bass_kernel_spmd

    V = int(V)
    s_len = np.asarray(x).shape[1]
    if s_len not in _cache:
        _cache[s_len] = _build(s_len)
    nc = _cache[s_len]

    ins = _host_inputs(x, Wq, Wk, Wv, Wo, V, s_len)
    kw = {}
    if _trace:
        kw = dict(trace=True)
    res = run_bass_kernel_spmd(nc, ins, core_ids=list(range(NCORES)), **kw)

    out = np.zeros((B, s_len, D), np.float32)
    for core in range(NCORES):
        out[core // 4] += res.results[core]["yp"]
    if _trace:
        kernel.last_result = res
    return out


# revision 24
# speedup vs baseline: 1.0434x; 1.0434x over previous
"""Distributed Trainium2 kernel for 8-head MHA with axial (2D) RoPE.

Problem: x:(2,4096,512) f32, Wq/Wk/Wv/Wo:(512,512), T=128, V=32.
  q/k/v = x @ W.T split into 8 heads of 64; q,k get axial rope
  (first 32 chans rotated by angle t_idx=s//V, next 32 by v_idx=s%V,
  interleaved-pair convention); dense softmax attention; out proj.

Sharding (8 cores): core c -> batch b=c//4, head pair (2*(c%4), 2*(c%4)+1).
Each core computes the full attention for its two heads and a partial
output projection over its 128 channels; the host sums the 4 partials
per batch.

Per-core kernel (matmuls bf16, accumulation f32):
  - host supplies x^T and W^T slices; rope is computed in the
    channels-on-partitions layout as
      q_rot = q^T*cosT + swap(q^T)*sinT_signed
    where swap() exchanges channel pairs via two partition-strided
    SBUF->SBUF DMAs and the cos/sin tables are host-baked per channel
    row -> three dense DVE ops, no transposes on the projection path.
  - attention: scores^T = k^T.T @ q^T per 128-key tile (both heads packed
    into one PE pass via row tile_position), exp on ScalarE straight from
    PSUM (scale=1/8 fused), PV matmul with a ones column appended to V so
    PSUM row 64 accumulates the softmax denominator. QK for tile t+2 is
    emitted before PV of tile t so ScalarE's exp stream never stalls on
    the in-order PE queue.
  - denominators transpose into [s-partition] layout via a DRAM bounce
    whose latency hides under the next chunk (final chunk: PE-mode
    transposes instead); reciprocal; applied per-partition after the
    out projection.
"""

import numpy as np
import ml_dtypes

B, S, D, H, HD = 2, 4096, 512, 8, 64
ROT_T = ROT_V = 32
ROPE_BASE = 10000.0
NCORES = 8
P = 128
CHUNK = 512  # sq chunk (one psum bank wide)

_cache = {}


def _install_drain_patch():
    """This walrus build allows only one sync-wait on a CTRL instruction;
    Tile's tail drain carries one wait per live semaphore. Move the extra
    waits onto dedicated SP nops."""
    import concourse.tile as tile
    import concourse.mybir as mybir
    from concourse.tile import ScopedClock

    if getattr(tile.TileContext, "_drain_patch_installed", False):
        return

    def _drain_and_barrier(self, tick_clock, wait_clock):
        nc = self.nc
        drain_inst = nc.sync.drain()
        wait_clock.add_sem_waits(
            drain_inst.ins, ScopedClock({None: tick_clock.global_clock})
        )
        si = drain_inst.ins.sync_info
        ow = list(si.on_wait or [])
        if len(ow) > 1:
            si.on_wait = [ow[0]]
            for w in ow[1:]:
                nop = nc.sync.nop(nofuse=True)
                nop.ins.sync_info = mybir.SyncInfo(on_wait=[w], on_update=[])
        nc.all_engine_barrier()
        popped = nc._tile_sem_poison_stack.pop()
        assert popped is self._sem_poison
        nc.clear_and_free_semaphores(list(self.sems.allocated().values()))
        nc.all_engine_barrier()

    tile.TileContext._drain_and_barrier = _drain_and_barrier
    tile.TileContext._drain_patch_installed = True


def _split_multiwaits(nc):
    """core_v3 walrus allows a single sync-wait command per instruction.
    Hoist extra waits onto same-engine NOPs inserted just before."""
    import concourse.mybir as mybir

    for f in nc.m.functions:
        for blk in f.blocks:
            new = []
            changed = False
            for ins in blk.instructions:
                si = getattr(ins, "sync_info", None)
                ow = list(si.on_wait) if (si is not None and si.on_wait) else []
                eng = getattr(ins, "engine", None)
                if len(ow) > 1 and eng is not None:
                    for i, w in enumerate(ow[:-1]):
                        new.append(
                            mybir.InstNoOp(
                                name=f"{ins.name}-sw{i}",
                                engine=eng,
                                sync_info=mybir.SyncInfo(
                                    on_wait=[w], on_update=[]
                                ),
                                bass_nofuse=True,
                            )
                        )
                    si.on_wait = [ow[-1]]
                    changed = True
                new.append(ins)
            if changed:
                blk.instructions = new


def _build(s_len):
    import concourse.bass as bass
    import concourse.tile as tile
    import concourse.mybir as mybir
    from concourse.bass import ds, ts

    _install_drain_patch()

    f32 = mybir.dt.float32
    bf16 = mybir.dt.bfloat16
    NT = s_len // P        # 128-row tiles (also key tiles)
    NCH = s_len // CHUNK   # 512-wide query chunks
    TPC = CHUNK // P       # s-tiles per chunk
    NG = NCH * NT          # total (chunk, key-tile) iterations

    nc = bass.Bass()
    xT = nc.dram_tensor("xT", [P, 4, s_len], bf16, kind="ExternalInput")
    wqT = nc.dram_tensor("wqT", [P, 4, P], bf16, kind="ExternalInput")
    wkT = nc.dram_tensor("wkT", [P, 4, P], bf16, kind="ExternalInput")
    wvT = nc.dram_tensor("wvT", [P, 4, P], bf16, kind="ExternalInput")
    woT = nc.dram_tensor("woT", [HD, 2, D], bf16, kind="ExternalInput")
    ctab = nc.dram_tensor("ctab", [P, s_len], bf16, kind="ExternalInput")
    stab = nc.dram_tensor("stab", [P, s_len], bf16, kind="ExternalInput")
    yp = nc.dram_tensor("yp", [s_len, D], f32, kind="ExternalOutput")

    i32 = mybir.dt.int32
    with tile.TileContext(nc) as tc:
        with (
            tc.tile_pool(name="const", bufs=1) as cpool,
            tc.tile_pool(name="pre", bufs=4) as pre,
            tc.tile_pool(name="vstg", bufs=2) as vstg,
            tc.tile_pool(name="expp", bufs=6) as expp,
            tc.tile_pool(name="schp", bufs=2) as schp,
            tc.tile_pool(name="dch", bufs=2) as dch,
            tc.tile_pool(name="outs", bufs=3) as outs,
            tc.tile_pool(name="qkps", bufs=2, space="PSUM") as qkps,
            tc.tile_pool(name="pvps", bufs=4, space="PSUM") as pvps,
            tc.tile_pool(name="dram", bufs=1, space="DRAM") as dram,
        ):
            # ---- constants. xT chunks 0-3 stream as dt-major 128KB
            # pieces in exactly the order the first projection group
            # consumes them (dt-outer, chunk-inner), so the matmuls chase
            # the DMA piece by piece; k weights + first table slices on
            # scalar; the rest via SWDGE ----
            wk_sb = cpool.tile([P, 4, P], bf16)
            nc.scalar.dma_start(wk_sb[:], wkT[:])
            xT_sb = cpool.tile([P, 4, s_len], bf16)
            for dt in range(4):
                for c in range(4):
                    nc.sync.dma_start(
                        xT_sb[:, dt, ts(c, CHUNK)], xT[:, dt, ts(c, CHUNK)]
                    )
            for c in range(4, NCH):
                nc.sync.dma_start(
                    xT_sb[:, :, ts(c, CHUNK)], xT[:, :, ts(c, CHUNK)]
                )
            half = 4 * CHUNK
            ct_sb = cpool.tile([P, s_len], bf16)
            st_sb = cpool.tile([P, s_len], bf16)
            nc.scalar.dma_start(ct_sb[:, 0:half], ctab[:, 0:half])
            nc.scalar.dma_start(st_sb[:, 0:half], stab[:, 0:half])
            nc.gpsimd.dma_start(
                ct_sb[:, half:s_len], ctab[:, half:s_len]
            )
            nc.gpsimd.dma_start(
                st_sb[:, half:s_len], stab[:, half:s_len]
            )
            wq_sb = cpool.tile([P, 4, P], bf16)
            nc.gpsimd.dma_start(wq_sb[:], wqT[:])
            wv_sb = cpool.tile([P, 4, P], bf16)
            nc.gpsimd.dma_start(wv_sb[:], wvT[:])
            wo_sb = cpool.tile([HD, 2, D], bf16)
            nc.gpsimd.dma_start(wo_sb[:], woT[:])

            # warm the PE clock (HAM) right before the projections: the
            # warmup matmuls read the first xT chunk so they execute just
            # after that DMA lands, not at kernel start
            wps = qkps.tile([P, 2, CHUNK], f32, tag="qk")
            for i in range(10):
                nc.tensor.matmul(
                    wps[:, 0, :],
                    lhsT=xT_sb[:, 0, 0:P],
                    rhs=xT_sb[:, 0, ts(0, CHUNK)],
                    start=(i == 0),
                    stop=(i == 9),
                )

            qT = cpool.tile([P, s_len], bf16)   # roped q^T
            kT = cpool.tile([P, s_len], bf16)
            v_sb = cpool.tile([P, NT, 2, HD + 1], bf16)  # v natural + ones
            yT0 = cpool.tile([HD, s_len], bf16)
            yT1 = cpool.tile([HD, s_len], bf16)
            den_dram = dram.tile([2, s_len], f32)

            nc.vector.memset(v_sb[:, :, :, HD : HD + 1], 1.0)
            one64 = cpool.tile([HD + 1, 1], f32)
            nc.vector.memset(one64[:], 1.0)

            mul = mybir.AluOpType.mult
            add = mybir.AluOpType.add
            exp_pre = mybir.ActivationFunctionType.Exp
            # dummy exp at t=0 pulls the ~1.3us ACT_TABLE_LOAD forward
            tiny = cpool.tile([1, 2], f32)
            nc.vector.memset(tiny[:], 0.0)
            tinyo = cpool.tile([1, 2], bf16)
            nc.scalar.activation(tinyo[:], tiny[:], exp_pre, scale=1.0)

            # ---- q/k projections + rope ----
            # dt-outer over groups of 4 chunks so consecutive matmuls share
            # the stationary weights (LDWEIGHTS pull-ahead amortizes);
            # rotation applied per chunk right after both projections land.
            def proj_group(w_sb, dst, chunks, pool, tag):
                group = []
                for c in chunks:
                    pgt = pool.tile([P, CHUNK], f32, tag=tag, name=f"pg{c}")
                    group.append((c, pgt))
                for dt in range(4):
                    for c, ps in group:
                        nc.tensor.matmul(
                            ps[:],
                            lhsT=w_sb[:, dt, :],
                            rhs=xT_sb[:, dt, ts(c, CHUNK)],
                            start=(dt == 0),
                            stop=(dt == 3),
                        )
                for c, ps in group:
                    # psum->bf16 cast on the (prologue-idle) ScalarE so
                    # DVE only carries the three rope multiplies
                    nc.scalar.copy(dst[:, ts(c, CHUNK)], ps[:])

            def rot_chunk(dst, p0, ps0, c):
                sl = ts(c, CHUNK)
                nc.vector.tensor_tensor(dst[:, sl], p0[:, sl], ct_sb[:, sl], mul)
                nc.vector.tensor_tensor(ps0[:, sl], ps0[:, sl], st_sb[:, sl], mul)
                nc.vector.tensor_tensor(dst[:, sl], dst[:, sl], ps0[:, sl], add)

            groups = [
                list(range(g, min(g + 4, NCH))) for g in range(0, NCH, 4)
            ]

            def qk_chain(w_sb, dst):
                p0 = pre.tile([P, s_len], bf16, tag="pre")
                ps0 = pre.tile([P, s_len], bf16, tag="pre")
                for chunks in groups:
                    proj_group(w_sb, p0, chunks, pvps, "pv")
                    # channel-pair swap (the rope cross term) via two
                    # partition-strided SBUF->SBUF DMAs
                    gsl = ds(chunks[0] * CHUNK, len(chunks) * CHUNK)
                    nc.scalar.dma_start(ps0[0:P:2, gsl], p0[1:P:2, gsl])
                    nc.scalar.dma_start(ps0[1:P:2, gsl], p0[0:P:2, gsl])
                    for c in chunks:
                        rot_chunk(dst, p0, ps0, c)

            qk_chain(wk_sb, kT)
            qk_chain(wq_sb, qT)

            # ---- attention helpers ----
            exp_f = mybir.ActivationFunctionType.Exp
            copy_f = mybir.ActivationFunctionType.Copy
            scale = HD ** -0.5
            qks = {}
            eps = {}

            def offloaded(g):
                c, t = divmod(g, NT)
                return c >= 1 and t in (4, 20, 26)

            def emit_qk(g):
                if g >= NG:
                    return
                c, t = divmod(g, NT)
                qk = qkps.tile([P, 2, CHUNK], f32, tag="qk")
                nc.tensor.matmul(
                    qk[:, 0, :],
                    lhsT=kT[0:HD, ts(t, P)],
                    rhs=qT[0:HD, ts(c, CHUNK)],
                    start=True,
                    stop=True,
                    tile_position=(0, 0),
                )
                nc.tensor.matmul(
                    qk[:, 1, :],
                    lhsT=kT[HD:P, ts(t, P)],
                    rhs=qT[HD:P, ts(c, CHUNK)],
                    start=True,
                    stop=True,
                    tile_position=(HD, 0),
                )
                qks[g] = qk
                if offloaded(g):
                    # Schraudolph exp on DVE, computed in the 2-pair QK
                    # prefetch shadow so PV(g) never waits on it:
                    # exp(s/8) ~= bitcast of int32(A*s + B); A folds the
                    # 1/8 score scale and log2(e), B carries the 127
                    # exponent bias minus the mantissa-linear correction
                    # (C=486411; ~2% rel err that mostly cancels in the
                    # softmax ratio). Takes ~8% of the exp stream off
                    # the bottleneck ScalarE at DVE-quiet tiles.
                    ti = schp.tile([P, 2, CHUNK], i32, tag="ti")
                    nc.vector.tensor_scalar(
                        ti[:], qk[:], 1512775.40, 1064866805.0, mul, add
                    )
                    ep = expp.tile([P, 2, CHUNK], bf16, tag="ep")
                    nc.vector.tensor_copy(ep[:], ti[:].bitcast(f32))
                    eps[g] = ep

            # ---- v projection -> natural layout (chunked DMA transpose);
            # chunk 0 up front, the rest interleaved into the first
            # attention chunk's tile loop ----
            def v_unit(c):
                ps = pvps.tile([P, CHUNK], f32, tag="pv")
                for dt in range(4):
                    nc.tensor.matmul(
                        ps[:],
                        lhsT=wv_sb[:, dt, :],
                        rhs=xT_sb[:, dt, ts(c, CHUNK)],
                        start=(dt == 0),
                        stop=(dt == 3),
                    )
                vs = vstg.tile([P, CHUNK], bf16, tag="vs")
                nc.vector.tensor_copy(vs[:], ps[:])
                vn = vstg.tile([P, TPC, P], bf16, tag="vn")
                nc.sync.dma_start(vn[:], vs[:], transpose=True)
                nc.vector.tensor_copy(
                    v_sb[:, c * TPC : (c + 1) * TPC, :, 0:HD],
                    vn.rearrange("p j (h c) -> p j h c", h=2),
                )

            for c in range(NCH):
                v_unit(c)
            emit_qk(0)
            emit_qk(1)

            def recip_of(rrs):
                rr = dch.tile([P, 2, TPC], f32, tag="rr")
                nc.vector.reciprocal(rr[:], rrs)
                return rr

            def outproj_st(c, rr, st, tail=False):
                """One s-tile of the deferred output projection of chunk c.
                In the tail (after the last exp) ScalarE is free and takes
                one of the two per-partition scales."""
                sg = c * TPC + st
                op0 = pvps.tile([P, CHUNK], f32, tag="pv")
                nc.tensor.matmul(
                    op0[:],
                    lhsT=yT0[:, ts(sg, P)],
                    rhs=wo_sb[:, 0, :],
                    start=True,
                    stop=True,
                )
                acc = outs.tile([P, D], f32, tag="acc")
                if tail:
                    nc.scalar.activation(
                        acc[:], op0[:], copy_f, scale=rr[:, 0, st : st + 1]
                    )
                else:
                    nc.vector.tensor_scalar_mul(
                        acc[:], op0[:], rr[:, 0, st : st + 1]
                    )
                op1 = pvps.tile([P, CHUNK], f32, tag="pv")
                nc.tensor.matmul(
                    op1[:],
                    lhsT=yT1[:, ts(sg, P)],
                    rhs=wo_sb[:, 1, :],
                    start=True,
                    stop=True,
                )
                tmp = outs.tile([P, D], f32, tag="tmp")
                nc.vector.tensor_scalar_mul(tmp[:], op1[:], rr[:, 1, st : st + 1])
                nc.vector.tensor_tensor(acc[:], acc[:], tmp[:], add)
                nc.sync.dma_start(yp[ts(sg, P), :], acc[:])

            def outproj_all(c, rrs):
                rr = recip_of(rrs)
                for st in range(TPC):
                    outproj_st(c, rr, st)

            # ---- attention main loop. QK is prefetched 2 tiles ahead so
            # ScalarE's exp stream never waits on the in-order PE queue.
            # Chunk c-1's output projection is interleaved into chunk c's
            # tile loop (reciprocal at tile 6, one s-tile every 2 tiles
            # from tile 8) so the denominator DMA round-trip latency and
            # the projection matmuls hide under the exp stream. ----
            spread = NT >= 8 + 2 * TPC
            pend = None
            for c in range(NCH):
                if pend is not None and not spread:
                    outproj_all(*pend)
                    pend = None
                pv0 = pvps.tile([P, CHUNK], f32, tag="pv")
                pv1 = pvps.tile([P, CHUNK], f32, tag="pv")
                rr = None
                for t in range(NT):
                    g = c * NT + t
                    qk = qks.pop(g)
                    if g in eps:
                        ep = eps.pop(g)
                    else:
                        ep = expp.tile([P, 2, CHUNK], bf16, tag="ep")
                        nc.scalar.activation(ep[:], qk[:], exp_f, scale=scale)
                    emit_qk(g + 2)
                    nc.tensor.matmul(
                        pv0[0 : HD + 1, :],
                        lhsT=v_sb[:, t, 0, :],
                        rhs=ep[:, 0, :],
                        start=(t == 0),
                        stop=(t == NT - 1),
                    )
                    nc.tensor.matmul(
                        pv1[0 : HD + 1, :],
                        lhsT=v_sb[:, t, 1, :],
                        rhs=ep[:, 1, :],
                        start=(t == 0),
                        stop=(t == NT - 1),
                    )
                    if pend is not None and spread:
                        if t == 6:
                            rr = recip_of(pend[1])
                        elif t >= 8 and t % 2 == 0 and (t - 8) // 2 < TPC:
                            outproj_st(pend[0], rr, (t - 8) // 2)
                            if (t - 8) // 2 == TPC - 1:
                                pend = None
                # y^T and denominators out of PSUM; denominator transpose
                # via a DRAM bounce (latency hidden by the deferral)
                nc.vector.tensor_copy(yT0[:, ts(c, CHUNK)], pv0[0:HD, :])
                nc.vector.tensor_copy(yT1[:, ts(c, CHUNK)], pv1[0:HD, :])
                dt_sb = dch.tile([HD + 1, 2, CHUNK], f32, tag="den")
                nc.vector.tensor_copy(dt_sb[HD : HD + 1, 0, :], pv0[HD : HD + 1, :])
                nc.vector.tensor_copy(dt_sb[HD : HD + 1, 1, :], pv1[HD : HD + 1, :])
                if spread and c == NCH - 1:
                    last_dt = dt_sb
                    continue
                nc.sync.dma_start(
                    den_dram[0:1, ts(c, CHUNK)], dt_sb[HD : HD + 1, 0, :]
                )
                nc.sync.dma_start(
                    den_dram[1:2, ts(c, CHUNK)], dt_sb[HD : HD + 1, 1, :]
                )
                rt = dch.tile([P, 2, TPC], f32, tag="rt")
                for h in range(2):
                    nc.sync.dma_start(
                        rt[:, h, :],
                        den_dram[h, ts(c, CHUNK)].rearrange("(t p) -> p t", p=P),
                    )
                pend = (c, rt[:])
            if spread:
                # final chunk: transpose the denominator rows on the PE
                # (no DRAM round-trip on the tail critical path)
                dps = pvps.tile([P, 2, TPC], f32, tag="pv")
                for h in range(2):
                    for st in range(TPC):
                        nc.tensor.transpose(
                            dps[:, h, st : st + 1],
                            last_dt[HD : HD + 1, h, ts(st, P)],
                            one64[HD : HD + 1, :],
                        )
                rrl = dch.tile([P, 2, TPC], f32, tag="rr")
                nc.vector.reciprocal(rrl[:], dps[:])
                for st in range(TPC):
                    outproj_st(NCH - 1, rrl, st, tail=True)
            else:
                outproj_all(*pend)
    _split_multiwaits(nc)
    return nc


def _host_inputs(x, Wq, Wk, Wv, Wo, V, s_len):
    """Build the 8 per-core input dicts."""
    bf = ml_dtypes.bfloat16
    x = np.asarray(x, np.float32)
    Wq = np.asarray(Wq, np.float32)
    Wk = np.asarray(Wk, np.float32)
    Wv = np.asarray(Wv, np.float32)
    Wo = np.asarray(Wo, np.float32)

    # rope tables in channel-on-partition layout [128, s]:
    #   row h*64+c: cos/sin of the angle for pair f=c//2 (t-axis for c<32,
    #   v-axis for c>=32); sin row signed: -sin even c, +sin odd c.
    s = np.arange(s_len)
    half_t, half_v = ROT_T // 2, ROT_V // 2
    inv_t = 1.0 / (ROPE_BASE ** (np.arange(half_t, dtype=np.float64) / half_t))
    inv_v = 1.0 / (ROPE_BASE ** (np.arange(half_v, dtype=np.float64) / half_v))
    ang_t = (s // V)[:, None] * inv_t[None, :]  # (s, 16)
    ang_v = (s % V)[:, None] * inv_v[None, :]
    ang = np.concatenate([ang_t, ang_v], axis=1)  # (s, 32) per-pair angle
    ang_ch = np.repeat(ang, 2, axis=1)  # (s, 64) per-channel
    sign = np.where(np.arange(HD) % 2 == 0, -1.0, 1.0)[None, :]
    cos_ch = np.cos(ang_ch)  # (s, 64)
    sin_ch = np.sin(ang_ch) * sign
    ctab = np.ascontiguousarray(np.tile(cos_ch, (1, 2)).T).astype(bf)  # [128, s]
    stab = np.ascontiguousarray(np.tile(sin_ch, (1, 2)).T).astype(bf)

    # channel pair swap for the rope cross term, applied to weight rows
    perm = np.arange(D) ^ 1

    xT = {}
    for b in range(B):
        t = x[b, :s_len].T.reshape(4, P, s_len).transpose(1, 0, 2)
        xT[b] = np.ascontiguousarray(t).astype(bf)

    def wslice(W, sl):
        return np.ascontiguousarray(
            W[sl, :].T.reshape(4, P, P).transpose(1, 0, 2)
        ).astype(bf)

    ins = []
    for core in range(NCORES):
        b = core // 4
        hb = (core % 4) * 2 * HD
        sl = slice(hb, hb + 2 * HD)
        woT = np.ascontiguousarray(
            Wo[:, sl].T.reshape(2, HD, D).transpose(1, 0, 2)
        ).astype(bf)
        ins.append(
            {
                "xT": xT[b],
                "wqT": wslice(Wq, sl),
                "wkT": wslice(Wk, sl),
                "wvT": wslice(Wv, sl),
                "woT": woT,
                "ctab": ctab,
                "stab": stab,
            }
        )
    return ins


def kernel(x, Wq, Wk, Wv, Wo, T, V, _trace=False):
    from concourse.bass_utils import run_bass_kernel_spmd

    V = int(V)
    s_len = np.asarray(x).shape[1]
    if s_len not in _cache:
        _cache[s_len] = _build(s_len)
    nc = _cache[s_len]

    ins = _host_inputs(x, Wq, Wk, Wv, Wo, V, s_len)
    kw = {}
    if _trace:
        kw = dict(trace=True)
    res = run_bass_kernel_spmd(nc, ins, core_ids=list(range(NCORES)), **kw)

    out = np.zeros((B, s_len, D), np.float32)
    for core in range(NCORES):
        out[core // 4] += res.results[core]["yp"]
    if _trace:
        kernel.last_result = res
    return out

